# revision 66
# baseline (speedup 1.0000x reference)
"""Trainium2 Bass kernel for GQA MHA prefill (S=2048, D=4096, H=32, KVH=8).

Tensor-parallel over heads on 8 cores (4 q heads + 1 kv head per core).

Causal fast path (v2):
  - All projections in bf16 (halves HBM traffic + SBUF vs f32r; same PE
    rate). x is resident per 512-seq block. nb0 runs c-major over
    [k, q0, q1, v] so compute starts while weights stream; q2/q3 of nb0
    are deferred into nb1's window (wq is staged in two head-pair
    halves so nb0 only needs half the wq bytes); nb1-3 run tile-major
    with 3 rotating PSUM banks. All stage-1 loads go through the two
    HWDGE queues round-robin so DMA service order tracks emission
    order.
  - RoPE runs entirely on DVE reading PSUM directly: the half-swap is
    two partition-offset tensor_tensor multiplies (no PE swap matmul,
    no PSUM-staging copies). cos/sin staged as [c;c], [-s;s] with the
    logit scale folded in.
  - v tiles transposed to [kpos, hd] via DMA XBAR transpose (bf16);
    ones-column via one upfront memset (softmax denominators come free
    as the 129th column of the p@v matmul).
  - SDPA per head in [k, q] layout; exp'd strips are packed at exact
    diagonal granularity (column kc*128 onward only) and contiguous in
    SBUF, so exp is tiled by et column (17 x 1024 per head, fills
    split at PSUM bank boundaries). Fill/exp jobs are pipelined across
    heads with a global cursor; p@v for qc interleaves with fills for
    qc+2 so neither PE nor Act waits. Causal masking of the diagonal
    128x128 block is a bf16 0/1 lower-tri multiply on DVE, scheduled a
    qc early. Each PSUM bank packs two 129-wide p@v accumulators plus
    the pair's bf16 oT transpose target (bitcast view), giving po and
    oT a ring of 2 banks alongside the 3-deep logits ring.
  - o tiles normalized on DVE (per-partition reciprocal), PE-transposed
    in bf16 (deferred one qc so the DVE norm latency hides under p@v),
    AllToAll'd to seq-sharded layout; lh gathered per head right after
    its collective. Each core then computes its 256 output rows
    against wo streamed as bf16 [128,2048] pair-tiles in CORDER
    (head-major) so stage-3 chunks unblock as each head's a2a lands.
  - Non-causal masks fall back to the v1 general kernel (mask applied
    as data via identity-matmul accumulation).
"""

import sys

import numpy as np

sys.path.insert(0, "/opt/trn_rl_repo")

S = 2048
D = 4096
H = 32
KVH = 8
HD = 128
NCORES = 8
HL = H // NCORES          # 4 local query heads
DL = HL * HD              # 512 local q dim
SQ = S // NCORES          # 256 output rows per core
KC = S // 128             # 16 key chunks
DC = D // 128             # 32 contraction chunks
NB = S // 512             # 4 seq blocks of 512
NEG = -1e9
VST = 144                 # v_sb column stride: 128 hd + 1 ones + 15 pad (32B align)
VST_G = 130               # general-path v stride (v1 kernel)

ET_W = [S - 128 * kc for kc in range(KC)]
ET_OFF = [sum(ET_W[:kc]) for kc in range(KC)]
ET_COLS = sum(ET_W)       # 17408
CORDER = [rr * HL + h for h in range(HL) for rr in range(NCORES)]

# exp is tiled by et COLUMN (strips are contiguous), 17 tiles x 1024:
# each tile = segments (kc, in-strip offset, width, pl offset)
ET_NT = ET_COLS // 1024   # 17


def _et_plan():
    tiles = []
    for j in range(ET_NT):
        lo, hi = j * 1024, (j + 1) * 1024
        segs = []
        for kc in range(KC):
            a = max(lo, ET_OFF[kc])
            b = min(hi, ET_OFF[kc] + ET_W[kc])
            if b > a:
                segs.append((kc, a - ET_OFF[kc], b - a, a - lo))
        tiles.append(segs)
    return tiles


ET_TILES = _et_plan()
# deepest et tile index pv(s) touches: col ET_OFF[s] + 127
ET_TCOV = [(ET_OFF[s] + 127) // 1024 for s in range(KC)]

_built = {}


def _build_causal(for_sim: bool = False):
    import concourse.bass as bass  # noqa: F401
    import concourse.mybir as mybir
    import concourse.tile as tile
    from concourse import bacc
    from concourse.masks import make_identity

    fp32 = mybir.dt.float32
    bf16 = mybir.dt.bfloat16
    AF = mybir.ActivationFunctionType
    OP = mybir.AluOpType

    nc = bacc.Bacc(
        "TRN2",
        target_bir_lowering=False,
        debug=False,
        num_devices=1 if for_sim else NCORES,
    )
    xT = nc.dram_tensor("xT", [NB, 128, DC, 512], bf16, kind="ExternalInput")
    wqaT = nc.dram_tensor("wqaT", [128, DC, 256], bf16, kind="ExternalInput")
    wqbT = nc.dram_tensor("wqbT", [128, DC, 256], bf16, kind="ExternalInput")
    wkT = nc.dram_tensor("wkT", [128, DC, HD], bf16, kind="ExternalInput")
    wvT = nc.dram_tensor("wvT", [128, DC, HD], bf16, kind="ExternalInput")
    cosT = nc.dram_tensor("cosT", [128, S], fp32, kind="ExternalInput")
    sinT = nc.dram_tensor("sinT", [128, S], fp32, kind="ExternalInput")
    woT = nc.dram_tensor("woT", [4, 128, DC, 1024], bf16, kind="ExternalInput")
    out = nc.dram_tensor("out", [SQ, D], fp32, kind="ExternalOutput")

    rg = [list(range(NCORES))]

    with tile.TileContext(nc) as tc:
        with (
            tc.tile_pool(name="const", bufs=1) as constp,
            tc.tile_pool(name="pers", bufs=1) as pers,
            tc.tile_pool(name="dram", bufs=1, space="DRAM") as dramp,
        ):
            ident = constp.tile([128, 128], fp32, tag="ident")
            make_identity(nc, ident[:])
            idb = constp.tile([128, 128], bf16, tag="idb")
            nc.scalar.copy(idb[:], ident[:])
            # 0/1 lower-tri mask (keep where q >= k i.e. f >= p)
            trimask = constp.tile([128, 128], bf16, tag="trimask")
            nc.gpsimd.memset(trimask[:], 1.0)
            nc.gpsimd.affine_select(
                out=trimask[:], in_=trimask[:],
                pattern=[[1, 128]],
                compare_op=OP.is_ge,
                fill=0.0,
                base=0,
                channel_multiplier=-1,
            )

            qT_sb = pers.tile([128, HL * S], bf16, tag="qT")
            kT_sb = pers.tile([128, S], bf16, tag="kT")
            v_sb = pers.tile([128, KC * VST], bf16, tag="v")

            a2a_in = [
                dramp.tile(
                    [NCORES * HD, SQ], bf16,
                    tag=f"a2a_in{h}", name=f"a2a_in{h}",
                )
                for h in range(HL)
            ]
            a2a_out = [
                dramp.tile(
                    [NCORES * HD, SQ], bf16,
                    tag=f"a2a_out{h}", name=f"a2a_out{h}",
                )
                for h in range(HL)
            ]

            # ---------------- Stage 1: projections + RoPE ----------------
            with (
                tc.tile_pool(name="s1w", bufs=1) as s1w,
                tc.tile_pool(name="s1x", bufs=1) as s1x,
                tc.tile_pool(name="tabs", bufs=1) as tabs,
                tc.tile_pool(name="rope", bufs=2) as ropep,
                tc.tile_pool(name="vcp", bufs=2) as vcp,
                tc.tile_pool(name="ps6", bufs=1, space="PSUM") as ps6,
                tc.tile_pool(name="psr", bufs=3, space="PSUM") as psr,
            ):
                wqa_sb = s1w.tile([128, DC * 256], bf16, tag="wqa")
                wqb_sb = s1w.tile([128, DC * 256], bf16, tag="wqb")
                wk_sb = s1w.tile([128, DC * HD], bf16, tag="wk")
                wv_sb = s1w.tile([128, DC * HD], bf16, tag="wv")
                c_sb = tabs.tile([128, S], fp32, tag="cos")
                s_sb = tabs.tile([128, S], fp32, tag="sin")
                xts = [
                    s1x.tile([128, DC * 512], bf16, tag=f"xt{i}", name=f"xt{i}")
                    for i in range(2)
                ]
                nc.vector.memset(v_sb[:], 1.0)

                _rr = [0]

                def _eng():
                    _rr[0] += 1
                    return nc.scalar if _rr[0] % 2 == 0 else nc.sync

                def load_xt(nb, g):
                    _eng().dma_start(
                        xts[nb % 2][:, g * 2048 : (g + 1) * 2048],
                        xT[nb, :, 4 * g : 4 * g + 4, :].rearrange(
                            "p c m -> p (c m)"
                        ),
                    )

                def load_wq(half, p):
                    sb = wqa_sb if half == 0 else wqb_sb
                    src = wqaT if half == 0 else wqbT
                    eng = _eng()
                    eng.dma_start(
                        sb[:, 2 * p * 256 : (2 * p + 2) * 256],
                        src[:, 2 * p : 2 * p + 2, :].rearrange("p c m -> p (c m)"),
                    )

                def q_lhs(c, m):
                    if m < 2:
                        return wqa_sb[:, c * 256 + m * 128 : c * 256 + (m + 1) * 128]
                    return wqb_sb[:, c * 256 + (m - 2) * 128 : c * 256 + (m - 1) * 128]

                def load_wk(q):
                    nc.sync.dma_start(
                        wk_sb[:, q * 8 * HD : (q + 1) * 8 * HD],
                        wkT[:, 8 * q : 8 * q + 8, :].rearrange("p c m -> p (c m)"),
                    )

                def load_wk_r(c0, c1):
                    nc.sync.dma_start(
                        wk_sb[:, c0 * HD : c1 * HD],
                        wkT[:, c0:c1, :].rearrange("p c m -> p (c m)"),
                    )

                def load_xt_r(nb, c0, c1):
                    _eng().dma_start(
                        xts[nb % 2][:, c0 * 512 : c1 * 512],
                        xT[nb, :, c0:c1, :].rearrange("p c m -> p (c m)"),
                    )

                def load_wv(q):
                    nc.sync.dma_start(
                        wv_sb[:, q * 8 * HD : (q + 1) * 8 * HD],
                        wvT[:, 8 * q : 8 * q + 8, :].rearrange("p c m -> p (c m)"),
                    )

                def load_tab(nb):
                    nc.sync.dma_start(
                        c_sb[:, nb * 512 : (nb + 1) * 512],
                        cosT[:, nb * 512 : (nb + 1) * 512],
                    )
                    nc.sync.dma_start(
                        s_sb[:, nb * 512 : (nb + 1) * 512],
                        sinT[:, nb * 512 : (nb + 1) * 512],
                    )

                def rope(dst, acc, col0):
                    # dst = acc*cfull + halfswap(acc)*sfull; rows 0:64 even
                    # comps, 64:128 odd; cfull=[c;c], sfull=[-s;s] host-staged.
                    t1 = ropep.tile([128, 512], fp32, tag="t1")
                    nc.vector.tensor_tensor(
                        t1[:], acc[:], c_sb[:, col0 : col0 + 512], OP.mult
                    )
                    t2 = ropep.tile([128, 512], fp32, tag="t2")
                    nc.vector.tensor_tensor(
                        t2[0:64, :], acc[64:128, :],
                        s_sb[0:64, col0 : col0 + 512], OP.mult,
                    )
                    nc.vector.tensor_tensor(
                        t2[64:128, :], acc[0:64, :],
                        s_sb[64:128, col0 : col0 + 512], OP.mult,
                    )
                    nc.vector.tensor_tensor(dst, t1[:], t2[:], OP.add)

                def do_v(nb, vacc):
                    vt = vcp.tile([128, 512], bf16, tag="vt")
                    nc.scalar.copy(vt[:], vacc[:])
                    for j in range(4):
                        kcg = nb * 4 + j
                        nc.sync.dma_start(
                            v_sb[:, kcg * VST : kcg * VST + 128],
                            vt[:, j * 128 : (j + 1) * 128],
                            transpose=True,
                        )

                # ---- nb0: c-major over [k, q0, q1, v] (q2/q3 deferred) ----
                sched = {
                    0: [lambda: load_wk(0),
                        lambda: nc.scalar.dma_start(
                            xts[0][:, 0:512],
                            xT[0, :, 0:1, :].rearrange("p c m -> p (c m)"),
                        ),
                        lambda: nc.scalar.dma_start(
                            xts[0][:, 512:2048],
                            xT[0, :, 1:4, :].rearrange("p c m -> p (c m)"),
                        ),
                        lambda: load_wq(0, 0), lambda: load_wv(0),
                        lambda: load_wq(0, 1), lambda: load_xt(0, 1)],
                    2: [lambda: load_xt(0, 2), lambda: load_wq(0, 2)],
                    3: [lambda: load_wk(1), lambda: load_wv(1)],
                    4: [lambda: load_wq(0, 3)],
                    6: [lambda: load_xt(0, 3), lambda: load_wq(0, 4)],
                    8: [lambda: load_wq(0, 5)],
                    10: [lambda: load_xt(0, 4), lambda: load_wq(0, 6)],
                    11: [lambda: load_wk(2), lambda: load_wv(2)],
                    12: [lambda: load_wq(0, 7)],
                    14: [lambda: load_xt(0, 5), lambda: load_wq(0, 8)],
                    16: [lambda: load_wq(0, 9)],
                    18: [lambda: load_xt(0, 6), lambda: load_wq(0, 10)],
                    19: [lambda: load_wk(3), lambda: load_wv(3)],
                    20: [lambda: load_wq(0, 11)],
                    22: [lambda: load_xt(0, 7), lambda: load_wq(0, 12)],
                    24: [lambda: load_wq(0, 13), lambda: load_wq(1, 0)],
                    26: [lambda: load_wq(0, 14), lambda: load_tab(0),
                         lambda: load_wq(1, 1)],
                    28: [lambda: load_wq(0, 15), lambda: load_wq(1, 2)],
                    29: [lambda: load_wq(1, 3)],
                    30: [lambda: load_wq(1, 4)],
                    31: [lambda: load_wq(1, 5)],
                }
                anames0 = ["kk", "q0", "q1", "vv"]
                accs = {
                    n: ps6.tile([128, 512], fp32, tag=n, name=n) for n in anames0
                }
                for c in range(DC):
                    for ld in sched.get(c, []):
                        ld()
                    st, sp = c == 0, c == DC - 1
                    xsl = xts[0][:, c * 512 : (c + 1) * 512]
                    nc.tensor.matmul(
                        accs["kk"][:], lhsT=wk_sb[:, c * HD : (c + 1) * HD],
                        rhs=xsl, start=st, stop=sp,
                    )
                    for m in range(2):
                        nc.tensor.matmul(
                            accs[f"q{m}"][:], lhsT=q_lhs(c, m),
                            rhs=xsl, start=st, stop=sp,
                        )
                    nc.tensor.matmul(
                        accs["vv"][:], lhsT=wv_sb[:, c * HD : (c + 1) * HD],
                        rhs=xsl, start=st, stop=sp,
                    )
                rope(kT_sb[:, 0:512], accs["kk"], 0)
                for m in range(2):
                    rope(qT_sb[:, m * S : m * S + 512], accs[f"q{m}"], 0)
                do_v(0, accs["vv"])

                # ---- remaining tiles, tile-major; per-tile load prefetch ----
                plan = [("q2", 0), ("q3", 0)]
                for nb in (1, 2, 3):
                    plan += [(t, nb) for t in ("kk", "q0", "q1", "q2", "q3", "vv")]
                loads = {
                    ("q2", 0): [lambda: load_wq(1, 6), lambda: load_wq(1, 7),
                                lambda: load_wq(1, 8), lambda: load_wq(1, 9),
                                lambda: load_wq(1, 10), lambda: load_wq(1, 11),
                                lambda: load_wq(1, 12), lambda: load_wq(1, 13),
                                lambda: load_wq(1, 14), lambda: load_wq(1, 15)],
                    ("q3", 0): [lambda: load_xt(1, 0), lambda: load_xt(1, 1),
                                lambda: load_xt(1, 2), lambda: load_xt(1, 3),
                                lambda: load_xt(1, 4)],
                    ("kk", 1): [lambda: load_xt(1, 5), lambda: load_xt(1, 6),
                                lambda: load_xt(1, 7), lambda: load_tab(1)],
                    ("q0", 1): [lambda: load_xt(2, 0), lambda: load_xt(2, 1)],
                    ("q1", 1): [lambda: load_xt(2, 2), lambda: load_xt(2, 3)],
                    ("q2", 1): [lambda: load_xt(2, 4), lambda: load_xt(2, 5)],
                    ("q3", 1): [lambda: load_xt(2, 6), lambda: load_xt(2, 7)],
                    ("vv", 1): [lambda: load_tab(2)],
                    ("q0", 2): [lambda: load_xt(3, 0), lambda: load_xt(3, 1)],
                    ("q1", 2): [lambda: load_xt(3, 2), lambda: load_xt(3, 3)],
                    ("q2", 2): [lambda: load_xt(3, 4), lambda: load_xt(3, 5)],
                    ("q3", 2): [lambda: load_xt(3, 6), lambda: load_xt(3, 7)],
                    ("vv", 2): [lambda: load_tab(3)],
                }
                for tn, nb in plan:
                    for ld in loads.get((tn, nb), []):
                        ld()
                    col0 = nb * 512
                    acc = psr.tile([128, 512], fp32, tag="acc")
                    for c in range(DC):
                        if tn == "kk":
                            lhs = wk_sb[:, c * HD : (c + 1) * HD]
                        elif tn == "vv":
                            lhs = wv_sb[:, c * HD : (c + 1) * HD]
                        else:
                            lhs = q_lhs(c, int(tn[1]))
                        nc.tensor.matmul(
                            acc[:], lhsT=lhs,
                            rhs=xts[nb % 2][:, c * 512 : (c + 1) * 512],
                            start=c == 0, stop=c == DC - 1,
                        )
                    if tn == "kk":
                        rope(kT_sb[:, col0 : col0 + 512], acc, col0)
                    elif tn == "vv":
                        do_v(nb, acc)
                    else:
                        m = int(tn[1])
                        rope(
                            qT_sb[:, m * S + col0 : m * S + col0 + 512],
                            acc, col0,
                        )

            # ---------------- Stage 2: SDPA per head + AllToAll ----------------
            with (
                tc.tile_pool(name="wo", bufs=12) as wop,
                tc.tile_pool(name="wolh", bufs=1) as wolh,
                tc.tile_pool(name="sd", bufs=2) as sd,
                tc.tile_pool(name="sds", bufs=2) as sds,
            ):
                lh_sb = wolh.tile([128, DC * SQ], bf16, tag="lh")
                lh4 = lh_sb.rearrange(
                    "p (rr hh q) -> p rr hh q", rr=NCORES, hh=HL
                )
                sdpa_ps = tc.tile_pool(name="ps_l", bufs=3, space="PSUM")
                ps_l = sdpa_ps.__enter__()
                sdpa_ps2 = tc.tile_pool(name="ps_o", bufs=2, space="PSUM")
                ps_o = sdpa_ps2.__enter__()
                # cross-head pipelined fill/exp jobs: global cursor over
                # (head, et tile) so Act never idles at head boundaries
                et_tiles = {}

                def get_et(h):
                    if h not in et_tiles:
                        et_tiles[h] = sd.tile(
                            [128, ET_COLS], bf16, tag="et", name=f"et{h}"
                        )
                    return et_tiles[h]

                jcur = [0]

                def emit_jobs_through(tgt):
                    while jcur[0] <= min(tgt, HL * ET_NT - 1):
                        g = jcur[0]
                        jcur[0] += 1
                        h, j = divmod(g, ET_NT)
                        et = get_et(h)
                        pl = ps_l.tile([128, 1024], fp32, tag="pl")
                        for (kc, s0, w, off) in ET_TILES[j]:
                            # fills split at 512-col cuts: a matmul's
                            # PSUM write must stay within one bank
                            a = off
                            while a < off + w:
                                b = min(off + w, (a // 512 + 1) * 512)
                                qg = kc * 128 + s0 + (a - off)
                                nc.tensor.matmul(
                                    pl[:, a:b],
                                    lhsT=kT_sb[:, kc * 128 : (kc + 1) * 128],
                                    rhs=qT_sb[:, h * S + qg : h * S + qg + b - a],
                                    start=True, stop=True,
                                )
                                a = b
                        nc.scalar.activation(
                            et[:, j * 1024 : (j + 1) * 1024],
                            pl[:, 0:1024], AF.Exp,
                        )

                def tile_target(h, qc):
                    if qc >= KC:
                        h, qc = h + 1, qc - KC
                    if h >= HL:
                        return HL * ET_NT - 1
                    return h * ET_NT + ET_TCOV[qc]

                mcur = [0]

                def emit_mults_through(tgt):
                    # zero upper triangle of each strip's diagonal block;
                    # scheduled one qc early so its exp is pipeline-old
                    while mcur[0] <= min(tgt, HL * KC - 1):
                        gm = mcur[0]
                        mcur[0] += 1
                        h, kc = divmod(gm, KC)
                        et = get_et(h)
                        nc.vector.tensor_tensor(
                            et[:, ET_OFF[kc] : ET_OFF[kc] + 128],
                            et[:, ET_OFF[kc] : ET_OFF[kc] + 128],
                            trimask[:], OP.mult,
                        )

                def mult_target(h, qc):
                    if qc >= KC:
                        h, qc = h + 1, qc - KC
                    if h >= HL:
                        return HL * KC - 1
                    return h * KC + qc

                emit_jobs_through(ET_TCOV[1])
                emit_mults_through(0)
                for h in range(HL):
                    et = get_et(h)

                    pots = {}

                    def flush_osb(qc, osb, h=h):
                        # deferred by one qc: osb is ready, no PE wait.
                        # oT lands in the same bank as its po pair (cols
                        # 258:386 bitcast to bf16), so po+otp ride a ring
                        # of 2 banks together.
                        otp = pots[qc // 2][:, 258:386].bitcast(bf16)
                        nc.tensor.transpose(
                            otp[:, (qc % 2) * 128 : (qc % 2 + 1) * 128],
                            osb[:], idb[:],
                        )
                        if qc % 2 == 1:
                            ots = sds.tile([128, 256], bf16, tag="ots", bufs=4)
                            nc.vector.tensor_copy(ots[:], otp[:, 0:256])
                            nc.sync.dma_start(
                                a2a_in[h][(qc // 2) * 128 : (qc // 2 + 1) * 128, :],
                                ots[:],
                            )

                    pend = None
                    for qc in range(KC):
                        if qc % 2 == 0:
                            # two 129-wide accumulators + the pair's oT
                            # packed in one bank
                            pots[qc // 2] = ps_o.tile(
                                [128, 386], fp32, tag="po", name="po", bufs=2
                            )
                        po = pots[qc // 2][:, (qc % 2) * 129 : (qc % 2) * 129 + 129]
                        for kc in range(qc + 1):
                            o0 = ET_OFF[kc] + (qc - kc) * 128
                            nc.tensor.matmul(
                                po,
                                lhsT=et[:, o0 : o0 + 128],
                                rhs=v_sb[:, kc * VST : kc * VST + 129],
                                start=kc == 0, stop=kc == qc,
                            )
                        rc = sds.tile([128, 1], fp32, tag="rc", bufs=4)
                        nc.vector.reciprocal(rc[:], po[:, 128:129])
                        osb = sds.tile([128, 128], bf16, tag="osb", bufs=4)
                        nc.vector.tensor_scalar_mul(osb[:], po[:, 0:128], rc[:])
                        emit_mults_through(mult_target(h, qc + 1))
                        if pend is not None:
                            flush_osb(qc - 1, pend)
                        pend = osb
                        emit_jobs_through(tile_target(h, qc + 2))
                    flush_osb(KC - 1, pend)
                    if for_sim:
                        # timing proxy: collective replaced by local DMA
                        nc.sync.dma_start(a2a_out[h][:], a2a_in[h][:])
                    else:
                        nc.gpsimd.collective_compute(
                            "AllToAll",
                            mybir.AluOpType.bypass,
                            replica_groups=rg,
                            ins=[a2a_in[h][:].opt()],
                            outs=[a2a_out[h][:].opt()],
                        )
                    nc.sync.dma_start(
                        lh4[:, :, h, :],
                        a2a_out[h].rearrange("(rr p) q -> p rr q", p=128),
                    )

                sdpa_ps2.__exit__(None, None, None)
                sdpa_ps.__exit__(None, None, None)
                # ------------- Stage 3: output projection -------------
                with (
                    tc.tile_pool(name="woob", bufs=4) as woob,
                    tc.tile_pool(name="ps_w", bufs=2, space="PSUM") as ps_w,
                ):
                    for nbog in range(4):
                        last = nbog == 3
                        pw = [
                            ps_w.tile([128, 512], fp32, tag=f"wo{m}", name=f"pw{m}")
                            for m in range(4)
                        ]
                        wts = []
                        for cp in range(DC // 2):
                            wt = wop.tile([128, 2048], bf16, tag="wt")
                            eng = nc.gpsimd if cp % 2 == 0 else nc.scalar
                            eng.dma_start(
                                wt[:],
                                woT[nbog, :, 2 * cp : 2 * cp + 2, :].rearrange(
                                    "p c m -> p (c m)"
                                ),
                            )
                            wts.append(wt)
                            for sub in range(2):
                                ci = 2 * cp + sub
                                if last and ci >= DC - 4:
                                    continue  # deferred to the m-major tail
                                c = CORDER[ci]
                                for m in range(4):
                                    nc.tensor.matmul(
                                        pw[m][:],
                                        lhsT=lh_sb[:, c * SQ + (m % 2) * 128 : c * SQ + (m % 2) * 128 + 128],
                                        rhs=wt[:, sub * 1024 + (m // 2) * 512 : sub * 1024 + (m // 2) * 512 + 512],
                                        start=(ci == 0),
                                        stop=(not last and ci == DC - 1),
                                    )
                        for m in range(4):
                            if last:
                                # finish this tile's last 4 chunks m-major so
                                # its store overlaps the next tile's matmuls
                                for ci in range(DC - 4, DC):
                                    c = CORDER[ci]
                                    nc.tensor.matmul(
                                        pw[m][:],
                                        lhsT=lh_sb[:, c * SQ + (m % 2) * 128 : c * SQ + (m % 2) * 128 + 128],
                                        rhs=wts[ci // 2][:, (ci % 2) * 1024 + (m // 2) * 512 : (ci % 2) * 1024 + (m // 2) * 512 + 512],
                                        start=False,
                                        stop=(ci == DC - 1),
                                    )
                            ob = woob.tile([128, 512], fp32, tag="ob")
                            if m % 2 == 0:
                                nc.scalar.copy(ob[:], pw[m][:])
                            else:
                                nc.vector.tensor_copy(ob[:], pw[m][:])
                            nc.sync.dma_start(
                                out[
                                    (m % 2) * 128 : (m % 2 + 1) * 128,
                                    (nbog * 2 + m // 2) * 512 : (nbog * 2 + m // 2 + 1) * 512,
                                ],
                                ob[:],
                            )
    nc.compile()
    return nc


_PERM = np.concatenate([np.arange(0, HD, 2), np.arange(1, HD, 2)])


def _stage_inputs_causal(x, wq, wk, wv, wo, freqs_cos, freqs_sin):
    import ml_dtypes

    bf = ml_dtypes.bfloat16
    alpha = float(HD) ** -0.25  # sqrt of logit scale folded into both ropes
    xTs = np.ascontiguousarray(
        x.reshape(NB, 512, DC, 128).transpose(0, 3, 2, 1)
    ).astype(bf)
    ct = freqs_cos.T * alpha
    st = freqs_sin.T * alpha
    cosTc = np.ascontiguousarray(
        np.concatenate([ct, ct], axis=0), dtype=np.float32
    )
    sinTc = np.ascontiguousarray(
        np.concatenate([-st, st], axis=0), dtype=np.float32
    )
    woTs = np.ascontiguousarray(
        wo.reshape(4, 1024, DC, 128).transpose(0, 3, 2, 1)[:, :, CORDER, :]
    ).astype(bf)
    in_maps = []
    for i in range(NCORES):
        wq_i = (
            wq[i * DL : (i + 1) * DL, :]
            .reshape(HL, HD, D)[:, _PERM, :]
            .reshape(DL, D)
        )
        wk_i = wk[i * HD : (i + 1) * HD, :][_PERM, :]
        wv_i = wv[i * HD : (i + 1) * HD, :]
        wqaTs = np.ascontiguousarray(
            wq_i[0:256].T.reshape(DC, 128, 256).transpose(1, 0, 2)
        ).astype(bf)
        wqbTs = np.ascontiguousarray(
            wq_i[256:512].T.reshape(DC, 128, 256).transpose(1, 0, 2)
        ).astype(bf)
        wkTs = np.ascontiguousarray(
            wk_i.T.reshape(DC, 128, HD).transpose(1, 0, 2)
        ).astype(bf)
        wvTs = np.ascontiguousarray(
            wv_i.T.reshape(DC, 128, HD).transpose(1, 0, 2)
        ).astype(bf)
        in_maps.append(
            dict(
                xT=xTs, wqaT=wqaTs, wqbT=wqbTs, wkT=wkTs, wvT=wvTs,
                cosT=cosTc, sinT=sinTc, woT=woTs,
            )
        )
    return in_maps


# ---------------------------------------------------------------------------
# v1 general path (arbitrary additive mask), kept as the fallback.
# ---------------------------------------------------------------------------


def _build_general(for_sim: bool = False):
    import concourse.bass as bass  # noqa: F401
    import concourse.mybir as mybir
    import concourse.tile as tile
    from concourse import bacc
    from concourse.masks import make_identity

    fp32 = mybir.dt.float32
    bf16 = mybir.dt.bfloat16
    AF = mybir.ActivationFunctionType
    OP = mybir.AluOpType

    nc = bacc.Bacc(
        "TRN2",
        target_bir_lowering=False,
        debug=False,
        num_devices=1 if for_sim else NCORES,
    )
    f32r = mybir.dt.float32r
    xT = nc.dram_tensor("xT", [DC, NB, 128, 512], f32r, kind="ExternalInput")
    wqT = nc.dram_tensor("wqT", [D, DL], f32r, kind="ExternalInput")
    wkT = nc.dram_tensor("wkT", [D, HD], f32r, kind="ExternalInput")
    wvT = nc.dram_tensor("wvT", [D, HD], f32r, kind="ExternalInput")
    cosT = nc.dram_tensor("cosT", [128, S], fp32, kind="ExternalInput")
    sinT = nc.dram_tensor("sinT", [128, S], fp32, kind="ExternalInput")
    woT = nc.dram_tensor("woT", [DC, 4, 128, 1024], bf16, kind="ExternalInput")
    maskT = nc.dram_tensor("maskT", [S, S], fp32, kind="ExternalInput")
    out = nc.dram_tensor("out", [SQ, D], fp32, kind="ExternalOutput")

    rg = [list(range(NCORES))]

    with tile.TileContext(nc) as tc:
        with (
            tc.tile_pool(name="const", bufs=1) as constp,
            tc.tile_pool(name="pers", bufs=1) as pers,
            tc.tile_pool(name="dram", bufs=1, space="DRAM") as dramp,
        ):
            ident = constp.tile([128, 128], fp32, tag="ident")
            make_identity(nc, ident[:])
            c_sb = constp.tile([128, S], fp32, tag="cos")
            s_sb = constp.tile([128, S], fp32, tag="sin")
            # half-swap permutation: (Psw^T x)[p] = x[(p+64) % 128]
            psw = constp.tile([128, 128], fp32, tag="psw")
            nc.gpsimd.memset(psw[:], 0.0)
            for b0 in (64, -64):
                nc.gpsimd.affine_select(
                    out=psw[:], in_=psw[:],
                    pattern=[[-1, 128]],
                    compare_op=OP.not_equal,
                    fill=1.0,
                    base=b0,
                    channel_multiplier=1,
                )
            pswr = constp.tile([128, 128], f32r, tag="pswr")
            nc.scalar.copy(pswr[:], psw[:])

            qT_sb = pers.tile([128, HL * S], f32r, tag="qT")
            kT_sb = pers.tile([128, S], f32r, tag="kT")
            v_sb = pers.tile([128, KC * VST_G], bf16, tag="v")

            a2a_in = [
                dramp.tile(
                    [NCORES * HD, SQ], bf16,
                    tag=f"a2a_in{h}", name=f"a2a_in{h}",
                )
                for h in range(HL)
            ]
            a2a_out = [
                dramp.tile(
                    [NCORES * HD, SQ], bf16,
                    tag=f"a2a_out{h}", name=f"a2a_out{h}",
                )
                for h in range(HL)
            ]

            # ---------------- Stage 1: projections + RoPE ----------------
            with (
                tc.tile_pool(name="s1w", bufs=1) as s1w,
                tc.tile_pool(name="s1x", bufs=10) as s1x,
                tc.tile_pool(name="rope", bufs=3) as ropep,
                tc.tile_pool(name="s1v", bufs=3) as s1v,
                tc.tile_pool(name="ps_q", bufs=1, space="PSUM") as ps_q,
                tc.tile_pool(name="ps_kv", bufs=1, space="PSUM") as ps_kv,
                tc.tile_pool(name="ps_tr", bufs=1, space="PSUM") as ps_tr,
                tc.tile_pool(name="ps_sw", bufs=1, space="PSUM") as ps_sw,
            ):
                wq_sb = s1w.tile([128, DC * DL], f32r, tag="wq")
                wk_sb = s1w.tile([128, DC * HD], f32r, tag="wk")
                wv_sb = s1w.tile([128, DC * HD], f32r, tag="wv")

                def load_kv_quarter(cq):
                    nc.sync.dma_start(
                        wk_sb[:, cq * 8 * HD : (cq + 1) * 8 * HD],
                        wkT[cq * 8 * 128 : (cq + 1) * 8 * 128, :].rearrange(
                            "(c p) m -> p c m", p=128
                        ),
                    )
                    nc.sync.dma_start(
                        wv_sb[:, cq * 8 * HD : (cq + 1) * 8 * HD],
                        wvT[cq * 8 * 128 : (cq + 1) * 8 * 128, :].rearrange(
                            "(c p) m -> p c m", p=128
                        ),
                    )

                def load_wq_pair(k):  # 512KB: covers c in {2k, 2k+1}
                    nc.scalar.dma_start(
                        wq_sb[:, k * 2 * DL : (k + 1) * 2 * DL],
                        wqT[k * 2 * 128 : (k + 1) * 2 * 128, :].rearrange(
                            "(c p) m -> p c m", p=128
                        ),
                    )

                load_kv_quarter(0)
                for k in range(3):
                    load_wq_pair(k)

                def rope(dst, stg, col0, ncol):
                    swp = ps_sw.tile([128, 512], fp32, tag="swp")
                    nc.tensor.matmul(
                        swp[:, 0:ncol], lhsT=pswr[:], rhs=stg[:, 0:ncol],
                        start=True, stop=True,
                    )
                    t1 = ropep.tile([128, 512], fp32, tag="t1")
                    nc.vector.tensor_tensor(
                        t1[:, 0:ncol], stg[:, 0:ncol].bitcast(fp32),
                        c_sb[:, col0 : col0 + ncol], OP.mult,
                    )
                    t2 = ropep.tile([128, 512], fp32, tag="t2")
                    nc.vector.tensor_tensor(
                        t2[:, 0:ncol], swp[:, 0:ncol],
                        s_sb[:, col0 : col0 + ncol], OP.mult,
                    )
                    nc.vector.tensor_tensor(
                        dst, t1[:, 0:ncol], t2[:, 0:ncol], OP.add
                    )

                for nb in range(NB):
                    qps = [
                        ps_q.tile([128, 512], fp32, tag=f"q{m}", name=f"q{m}")
                        for m in range(HL)
                    ]
                    kps = ps_kv.tile([128, 512], fp32, tag="kk")
                    vps = ps_kv.tile([128, 512], fp32, tag="vv")
                    for c in range(DC):
                        if nb == 0:
                            if c % 2 == 0 and 2 <= c <= 26:
                                load_wq_pair(c // 2 + 2)
                            if c in (6, 14, 22):
                                load_kv_quarter(c // 8 + 1)
                            if c == 3:
                                nc.sync.dma_start(c_sb[:], cosT[:, :])
                                nc.sync.dma_start(s_sb[:], sinT[:, :])
                        xt = s1x.tile([128, 512], f32r, tag="xt")
                        nc.gpsimd.dma_start(xt[:], xT[c, nb, :, :])
                        st = c == 0
                        sp = c == DC - 1
                        for m in range(HL):
                            nc.tensor.matmul(
                                qps[m][:],
                                lhsT=wq_sb[:, c * DL + m * 128 : c * DL + (m + 1) * 128],
                                rhs=xt[:],
                                start=st,
                                stop=sp,
                            )
                        nc.tensor.matmul(
                            kps[:],
                            lhsT=wk_sb[:, c * HD : (c + 1) * HD],
                            rhs=xt[:],
                            start=st,
                            stop=sp,
                        )
                        nc.tensor.matmul(
                            vps[:],
                            lhsT=wv_sb[:, c * HD : (c + 1) * HD],
                            rhs=xt[:],
                            start=st,
                            stop=sp,
                        )
                    stk = s1v.tile([128, 512], f32r, tag="stq")
                    nc.scalar.copy(stk[:], kps[:])
                    rope(kT_sb[:, nb * 512 : (nb + 1) * 512], stk, nb * 512, 512)
                    for m in range(HL):
                        stg = s1v.tile([128, 512], f32r, tag="stq")
                        nc.scalar.copy(stg[:], qps[m][:])
                        rope(
                            qT_sb[:, m * S + nb * 512 : m * S + (nb + 1) * 512],
                            stg,
                            nb * 512,
                            512,
                        )
                    vt = s1v.tile([128, 512], fp32, tag="vt")
                    nc.scalar.copy(vt[:], vps[:])
                    for j in range(4):
                        kcg = nb * 4 + j
                        vtp = ps_tr.tile([128, 128], fp32, tag="vtr")
                        nc.tensor.transpose(
                            vtp[:], vt[:, j * 128 : (j + 1) * 128], ident[:]
                        )
                        nc.scalar.copy(
                            v_sb[:, kcg * VST_G : kcg * VST_G + 128], vtp[:]
                        )
                        nc.vector.memset(
                            v_sb[:, kcg * VST_G + 128 : kcg * VST_G + 129], 1.0
                        )

            # ---------------- Stage 2: SDPA per head + AllToAll ----------------
            with (
                tc.tile_pool(name="wo", bufs=11) as wop,
                tc.tile_pool(name="wolh", bufs=1) as wolh,
                tc.tile_pool(name="sd", bufs=2) as sd,
                tc.tile_pool(name="sds", bufs=2) as sds,
                tc.tile_pool(name="msk", bufs=4) as mskp,
            ):
                et_lo = [0 for kc in range(KC)]
                et_w = [S - lo for lo in et_lo]
                et_off = [sum(et_w[:kc]) for kc in range(KC)]
                et_cols = sum(et_w)
                sdpa_ps = tc.tile_pool(name="ps_l", bufs=2, space="PSUM")
                ps_l = sdpa_ps.__enter__()
                sdpa_ps2 = tc.tile_pool(name="ps_o", bufs=2, space="PSUM")
                ps_o = sdpa_ps2.__enter__()
                for h in range(HL):
                    et = sd.tile([128, et_cols], bf16, tag="et", bufs=1)
                    for kc in range(KC):
                        q_lo = et_lo[kc]
                        for t0 in range(q_lo, S, 1024):
                            width = min(1024, S - t0)
                            pl = ps_l.tile([128, 1024], fp32, tag="pl")
                            qbs = range(t0 // 512, (t0 + width) // 512)
                            for qb in qbs:
                                fo = qb * 512 - t0
                                mt = mskp.tile([128, 512], fp32, tag="mt")
                                nc.sync.dma_start(
                                    mt[:],
                                    maskT[
                                        kc * 128 : (kc + 1) * 128,
                                        qb * 512 : (qb + 1) * 512,
                                    ],
                                )
                                nc.tensor.matmul(
                                    pl[:, fo : fo + 512],
                                    lhsT=ident[:],
                                    rhs=mt[:],
                                    start=True,
                                    stop=False,
                                )
                            for qb in qbs:
                                fo = qb * 512 - t0
                                nc.tensor.matmul(
                                    pl[:, fo : fo + 512],
                                    lhsT=kT_sb[:, kc * 128 : (kc + 1) * 128],
                                    rhs=qT_sb[:, h * S + qb * 512 : h * S + (qb + 1) * 512],
                                    start=False,
                                    stop=True,
                                )
                            e0 = t0
                            base = et_off[kc] - q_lo
                            nc.scalar.activation(
                                et[:, base + e0 : base + t0 + width],
                                pl[:, e0 - t0 : width],
                                AF.Exp,
                            )
                    for qc in range(KC):
                        kc_hi = KC - 1
                        po = ps_o.tile([128, 129], fp32, tag="po")
                        for kc in range(kc_hi + 1):
                            nc.tensor.matmul(
                                po[:],
                                lhsT=et[
                                    :,
                                    et_off[kc] - et_lo[kc] + qc * 128 : et_off[kc]
                                    - et_lo[kc]
                                    + qc * 128
                                    + 128,
                                ],
                                rhs=v_sb[:, kc * VST_G : kc * VST_G + 129],
                                start=(kc == 0),
                                stop=(kc == kc_hi),
                            )
                        rc = sds.tile([128, 1], fp32, tag="rc")
                        nc.vector.reciprocal(rc[:], po[:, 128:129])
                        osb = sds.tile([128, 128], fp32, tag="osb")
                        nc.vector.tensor_scalar_mul(osb[:], po[:, 0:128], rc[:])
                        otp = ps_o.tile([128, 129], fp32, tag="po", name="otp")
                        nc.tensor.transpose(otp[:, 0:128], osb[:], ident[:])
                        if qc % 2 == 0:
                            ots = sds.tile([128, 256], bf16, tag="ots")
                        nc.vector.tensor_copy(
                            ots[:, (qc % 2) * 128 : (qc % 2 + 1) * 128],
                            otp[:, 0:128],
                        )
                        if qc % 2 == 1:
                            nc.sync.dma_start(
                                a2a_in[h][(qc // 2) * 128 : (qc // 2 + 1) * 128, :],
                                ots[:],
                            )
                    if for_sim:
                        nc.sync.dma_start(a2a_out[h][:], a2a_in[h][:])
                    else:
                        nc.gpsimd.collective_compute(
                            "AllToAll",
                            OP.bypass,
                            replica_groups=rg,
                            ins=[a2a_in[h][:].opt()],
                            outs=[a2a_out[h][:].opt()],
                        )

                sdpa_ps2.__exit__(None, None, None)
                sdpa_ps.__exit__(None, None, None)
                # ------------- Stage 3: output projection -------------
                with (
                    tc.tile_pool(name="woob", bufs=2) as woob,
                    tc.tile_pool(name="ps_w", bufs=2, space="PSUM") as ps_w,
                ):
                    lh_sb = wolh.tile([128, DC * SQ], bf16, tag="lh")
                    lh4 = lh_sb.rearrange(
                        "p (rr hh q) -> p rr hh q", rr=NCORES, hh=HL
                    )
                    for h in range(HL):
                        nc.sync.dma_start(
                            lh4[:, :, h, :],
                            a2a_out[h].rearrange("(rr p) q -> p rr q", p=128),
                        )
                    corder = [rr * HL + h for h in range(HL) for rr in range(NCORES)]
                    for nbog in range(4):
                        pw = [
                            ps_w.tile([128, 512], fp32, tag=f"wo{m}", name=f"pw{m}")
                            for m in range(4)
                        ]
                        for ci, c in enumerate(corder):
                            wt = wop.tile([128, 1024], bf16, tag="wt")
                            dma_eng = nc.gpsimd if ci % 2 == 0 else nc.scalar
                            dma_eng.dma_start(wt[:], woT[c, nbog, :, :])
                            for m in range(4):
                                nc.tensor.matmul(
                                    pw[m][:],
                                    lhsT=lh_sb[:, c * SQ + (m % 2) * 128 : c * SQ + (m % 2 + 1) * 128],
                                    rhs=wt[:, (m // 2) * 512 : (m // 2 + 1) * 512],
                                    start=(ci == 0),
                                    stop=(ci == DC - 1),
                                )
                        for m in range(4):
                            ob = woob.tile([128, 512], fp32, tag="ob")
                            nc.vector.tensor_copy(ob[:], pw[m][:])
                            nc.sync.dma_start(
                                out[
                                    (m % 2) * 128 : (m % 2 + 1) * 128,
                                    (nbog * 2 + m // 2) * 512 : (nbog * 2 + m // 2 + 1) * 512,
                                ],
                                ob[:],
                            )
    nc.compile()
    return nc


def _build(causal: bool, for_sim: bool = False):
    if causal:
        return _build_causal(for_sim)
    return _build_general(for_sim)


def _stage_inputs_general(x, wq, wk, wv, wo, mask, freqs_cos, freqs_sin):
    alpha = float(HD) ** -0.25
    import ml_dtypes

    xTc = np.ascontiguousarray(
        x.T.reshape(DC, 128, NB, 512).transpose(0, 2, 1, 3)
    )
    woTc = np.ascontiguousarray(
        wo.T.reshape(DC, 128, 4, 1024).transpose(0, 2, 1, 3)
    ).astype(ml_dtypes.bfloat16)
    ct = freqs_cos.T * alpha
    st = freqs_sin.T * alpha
    cosTc = np.ascontiguousarray(np.concatenate([ct, ct], axis=0))
    sinTc = np.ascontiguousarray(np.concatenate([-st, st], axis=0))
    maskTc = np.ascontiguousarray(np.maximum(mask, -60.0).T)
    in_maps = []
    for i in range(NCORES):
        wq_i = wq[i * DL : (i + 1) * DL, :].reshape(HL, HD, D)[:, _PERM, :]
        wk_i = wk[i * HD : (i + 1) * HD, :][_PERM, :]
        wv_i = wv[i * HD : (i + 1) * HD, :]
        m = dict(
            xT=xTc,
            wqT=np.ascontiguousarray(wq_i.reshape(DL, D).T),
            wkT=np.ascontiguousarray(wk_i.T),
            wvT=np.ascontiguousarray(wv_i.T),
            cosT=cosTc,
            sinT=sinTc,
            woT=woTc,
            maskT=maskTc,
        )
        in_maps.append(m)
    return in_maps


def _is_causal(mask):
    if mask.shape != (S, S):
        return False
    tri = np.tril(np.ones((S, S), bool))
    return bool(
        np.all(mask[tri] == 0.0) and np.all(mask[~tri] <= -1e8)
    )


def run(inputs, trace=False):
    from concourse.bass_utils import run_bass_kernel_spmd

    causal = _is_causal(np.asarray(inputs["mask"]))
    if causal not in _built:
        _built[causal] = _build(causal)
    nc = _built[causal]
    if causal:
        in_maps = _stage_inputs_causal(
            np.asarray(inputs["x"], np.float32),
            np.asarray(inputs["wq"], np.float32),
            np.asarray(inputs["wk"], np.float32),
            np.asarray(inputs["wv"], np.float32),
            np.asarray(inputs["wo"], np.float32),
            np.asarray(inputs["freqs_cos"], np.float32),
            np.asarray(inputs["freqs_sin"], np.float32),
        )
    else:
        in_maps = _stage_inputs_general(
            np.asarray(inputs["x"], np.float32),
            np.asarray(inputs["wq"], np.float32),
            np.asarray(inputs["wk"], np.float32),
            np.asarray(inputs["wv"], np.float32),
            np.asarray(inputs["wo"], np.float32),
            np.asarray(inputs["mask"], np.float32),
            np.asarray(inputs["freqs_cos"], np.float32),
            np.asarray(inputs["freqs_sin"], np.float32),
        )
    res = run_bass_kernel_spmd(
        nc, in_maps, core_ids=list(range(NCORES)), trace=trace
    )
    out = np.concatenate([res.results[i]["out"] for i in range(NCORES)], axis=0)
    return out, res


def kernel(**inputs):
    out, _ = run(inputs, trace=False)
    return out
